# revision 8
# baseline (speedup 1.0000x reference)
"""PointNet++ Feature Propagation module on 8 Trainium2 NeuronCores.

Data-parallel over batch: B=16 -> 2 batches per core.
Per batch on-device: exact squared-distance matrix via split-precision f32r
matmuls (K=13 augmented), exact top-3 via DVE Max/MaxIndex reading PSUM,
inverse-distance weights on DVE, neighbor gather via SWDGE dma_gather,
weighted combine + transpose back via PSUM-accumulating PE transposes,
2-layer MLP in f32r with training-mode BatchNorm whose statistics are
all-reduced across the 8 cores in-kernel.
"""
import sys

sys.path.insert(0, "/opt/trn_rl_repo")

import numpy as np

import concourse.bacc as bacc
import concourse.tile as tile
from concourse import mybir
from concourse.bass_utils import run_bass_kernel_spmd

F32 = mybir.dt.float32
F32R = mybir.dt.float32r
U16 = mybir.dt.uint16
I16 = mybir.dt.int16
AF = mybir.ActivationFunctionType
ALU = mybir.AluOpType

B, N1, N2, C1, C2 = 16, 4096, 1024, 128, 256
CIN, CH, COUT = C1 + C2, 256, 256
NCORES = 8
BPC = B // NCORES          # batches per core
KAUG = 16                  # augmented K rows (13 used)
NT = N1 // 128             # 32 point tiles per batch
NG = N1 // 512             # 8 point groups per batch
BN_EPS = 1e-5
NTOT = float(B * N1)       # batchnorm count

_NC_CACHE = {}


def _raw_max(nc, out_ap, in_ap):
    nc.vector.add_instruction(
        mybir.InstMax(
            name=nc.get_next_instruction_name(),
            ins=[nc.vector.lower_ap(in_ap)],
            outs=[nc.vector.lower_ap(out_ap)],
        )
    )


def _raw_max_index(nc, out_ap, in_max_ap, in_values_ap):
    nc.vector.add_instruction(
        mybir.InstMaxIndex(
            name=nc.get_next_instruction_name(),
            ins=[nc.vector.lower_ap(in_max_ap), nc.vector.lower_ap(in_values_ap)],
            outs=[nc.vector.lower_ap(out_ap)],
        )
    )


def build(ncores_cc=NCORES):
    if "nc" in _NC_CACHE:
        return _NC_CACHE["nc"]
    nc = bacc.Bacc("TRN2", target_bir_lowering=False, debug=False,
                   num_devices=NCORES)

    x1aug = nc.dram_tensor("x1aug", [BPC, KAUG, N1], F32R, kind="ExternalInput").ap()
    x2aug = nc.dram_tensor("x2aug", [BPC, KAUG, N2], F32R, kind="ExternalInput").ap()
    p1 = nc.dram_tensor("p1", [BPC, C1, N1], F32R, kind="ExternalInput").ap()
    p2t = nc.dram_tensor("p2t", [BPC, N2, C2], F32, kind="ExternalInput").ap()
    w1t = nc.dram_tensor("w1t", [128, 3, CH], F32R, kind="ExternalInput").ap()
    w2t = nc.dram_tensor("w2t", [128, 2, COUT], F32R, kind="ExternalInput").ap()
    identw = nc.dram_tensor("identw", [128, 128], F32R, kind="ExternalInput").ap()
    bnp = nc.dram_tensor("bnp", [128, 8], F32, kind="ExternalInput").ap()
    out = nc.dram_tensor("out", [BPC, COUT, N1], F32, kind="ExternalOutput").ap()

    stage = nc.dram_tensor("stage", [BPC, 16, NT * 8, 3], U16).ap()
    y2d = nc.dram_tensor("y2d", [BPC, 2, NG, 128, 512], F32).ap()
    cc_in = [nc.dram_tensor(f"cc_in{l}", [128, 4], F32).ap() for l in range(2)]
    cc_out = [
        nc.dram_tensor(f"cc_out{l}", [128, 4], F32, addr_space="Shared").ap()
        for l in range(2)
    ]

    with tile.TileContext(nc) as tc:
        import contextlib

        ctx = contextlib.ExitStack()
        with ctx:
            const = ctx.enter_context(tc.tile_pool(name="const", bufs=1))
            dist_ps = ctx.enter_context(tc.tile_pool(name="dist_ps", bufs=2, space="PSUM"))
            tp_ps = ctx.enter_context(tc.tile_pool(name="tp_ps", bufs=1, space="PSUM"))
            y_ps = ctx.enter_context(tc.tile_pool(name="y_ps", bufs=2, space="PSUM"))
            gpool = ctx.enter_context(tc.tile_pool(name="gpool", bufs=2))
            gspool = ctx.enter_context(tc.tile_pool(name="gspool", bufs=6))
            xpool = ctx.enter_context(tc.tile_pool(name="xpool", bufs=3))
            hpool = ctx.enter_context(tc.tile_pool(name="hpool", bufs=3))
            sqpool = ctx.enter_context(tc.tile_pool(name="sqpool", bufs=3))
            opool = ctx.enter_context(tc.tile_pool(name="opool", bufs=3))
            smpool = ctx.enter_context(tc.tile_pool(name="smpool", bufs=4))

            # ---- persistent/static tiles ----
            x1_sb, x2_sb, vall, iall, wall, idxw, y1_sb = [], [], [], [], [], [], []
            for b in range(BPC):
                x1_sb.append(const.tile([KAUG, N1], F32R, tag=f"x1sb{b}", name=f"x1sb{b}"))
                nc.sync.dma_start(x1_sb[b][:], x1aug[b])
                x2_sb.append(const.tile([KAUG, N2], F32R, tag=f"x2sb{b}", name=f"x2sb{b}"))
                nc.sync.dma_start(x2_sb[b][:], x2aug[b])
                vall.append(const.tile([128, NT, 8], F32, tag=f"vall{b}", name=f"vall{b}"))
                iall.append(const.tile([128, NT, 8], U16, tag=f"iall{b}", name=f"iall{b}"))
                wall.append(const.tile([128, NT, 3], F32, tag=f"wall{b}", name=f"wall{b}"))
                idxw.append(const.tile([128, 3, NT * 8], I16, tag=f"idxw{b}", name=f"idxw{b}"))
                y1_sb.append(const.tile([128, 2, N1], F32, tag=f"y1sb{b}", name=f"y1sb{b}"))
            w1t_sb = const.tile([128, 3, CH], F32R)
            nc.sync.dma_start(w1t_sb[:], w1t[:])
            w2t_sb = const.tile([128, 2, COUT], F32R)
            nc.sync.dma_start(w2t_sb[:], w2t[:])
            ident = const.tile([128, 128], F32R)
            nc.sync.dma_start(ident[:], identw[:])
            bnp_sb = const.tile([128, 8], F32)
            nc.sync.dma_start(bnp_sb[:], bnp[:])
            sums = const.tile([128, 2, 2, BPC, NG], F32)   # [layer, mh, b, g]
            sqs = const.tile([128, 2, 2, BPC, NG], F32)
            cc_sb = [const.tile([128, 4], F32, tag=f"cc{l}", name=f"cc{l}") for l in range(2)]
            svec = [const.tile([128, 2], F32, tag=f"sv{l}", name=f"sv{l}") for l in range(2)]
            bvec = [const.tile([128, 2], F32, tag=f"bv{l}", name=f"bv{l}") for l in range(2)]

            # =================== PHASE A ===================
            for b in range(BPC):
                for g in range(NG):
                    for t in range(4):
                        T = g * 4 + t
                        pd = dist_ps.tile([128, N2], F32, tag="pd")
                        lhs = x1_sb[b][:, T * 128:(T + 1) * 128]
                        for half in range(2):
                            nc.tensor.matmul(
                                pd[:, half * 512:(half + 1) * 512], lhs,
                                x2_sb[b][:, half * 512:(half + 1) * 512],
                                start=True, stop=True,
                            )
                        _raw_max(nc, vall[b][:, T, :], pd[:])
                        _raw_max_index(nc, iall[b][:, T, :], vall[b][:, T, :], pd[:])

                    # ---- weights math for the 4 tiles of this group ----
                    ts = slice(g * 4, g * 4 + 4)
                    dtmp = smpool.tile([128, 4, 3], F32, tag="dtmp")
                    # d = max(-v, 1e-10)
                    nc.vector.tensor_scalar_mul(dtmp[:], vall[b][:, ts, 0:3], -1.0)
                    nc.vector.tensor_scalar_max(dtmp[:], dtmp[:], 1e-10)
                    wtmp = smpool.tile([128, 4, 3], F32, tag="wtmp")
                    nc.vector.reciprocal(wtmp[:], dtmp[:])
                    ztmp = smpool.tile([128, 4], F32, tag="ztmp")
                    nc.vector.tensor_add(ztmp[:], wtmp[:, :, 0], wtmp[:, :, 1])
                    nc.vector.tensor_add(ztmp[:], ztmp[:], wtmp[:, :, 2])
                    zr = smpool.tile([128, 4], F32, tag="zr")
                    nc.vector.reciprocal(zr[:], ztmp[:])
                    nc.vector.tensor_tensor(
                        wall[b][:, ts, :], wtmp[:],
                        zr[:, :, None].to_broadcast([128, 4, 3]), ALU.mult)

                # ---- index staging to DRAM (8 partition-block DMAs) ----
                for a in range(8):
                    nc.sync.dma_start(
                        out=stage[b][:, a::8, :],
                        in_=iall[b][16 * a:16 * (a + 1), :, 0:3])
                # ---- wrapped index loads (8 replica blocks x 3 k) ----
                for k in range(3):
                    for gblk in range(8):
                        nc.sync.dma_start(
                            out=idxw[b][16 * gblk:16 * (gblk + 1), k, :],
                            in_=stage[b][:, :, k].bitcast(I16))

                # ---- gather + combine + transpose + x-assembly + MLP1 ----
                for g in range(NG):
                    gk = [gpool.tile([128, 4, C2], F32, tag=f"g{k}", name=f"g{k}") for k in range(3)]
                    for k in range(3):
                        nc.gpsimd.dma_gather(
                            out_ap=gk[k][:], in_ap=p2t[b],
                            idxs_ap=idxw[b][:, k, 32 * g:32 * (g + 1)],
                            num_idxs=512, num_idxs_reg=512, elem_size=C2)
                    xt = xpool.tile([128, 3, 512], F32R, tag="xt")
                    nc.sync.dma_start(xt[:, 0, :], p1[b][:, 512 * g:512 * (g + 1)])
                    tp = [tp_ps.tile([128, 512], F32R, tag=f"tp{h}", name=f"tp{h}") for h in range(2)]
                    for t in range(4):
                        T = g * 4 + t
                        for k in range(3):
                            gs = gspool.tile([128, C2], F32R, tag="gs")
                            nc.vector.tensor_scalar_mul(
                                gs[:], gk[k][:, t, :], wall[b][:, T, k:k + 1])
                            for half in range(2):
                                nc.tensor.matmul(
                                    tp[half][:, t * 128:(t + 1) * 128],
                                    gs[:, half * 128:(half + 1) * 128],
                                    ident[:], is_transpose=True,
                                    start=(k == 0), stop=(k == 2))
                    for half in range(2):
                        nc.scalar.activation(xt[:, 1 + half, :], tp[half][:], AF.Copy)
                    # MLP layer 1 for this group
                    for mh in range(2):
                        yp = y_ps.tile([128, 512], F32, tag="yp")
                        for kt in range(3):
                            nc.tensor.matmul(
                                yp[:], w1t_sb[:, kt, mh * 128:(mh + 1) * 128],
                                xt[:, kt, :], start=(kt == 0), stop=(kt == 2))
                        ysl = y1_sb[b][:, mh, 512 * g:512 * (g + 1)]
                        nc.scalar.activation(ysl, yp[:], AF.Copy)
                        nc.vector.tensor_reduce(
                            out=sums[:, 0, mh, b, g:g + 1], in_=ysl,
                            axis=mybir.AxisListType.X, op=ALU.add)
                        sq = sqpool.tile([128, 512], F32, tag="sq")
                        nc.vector.tensor_tensor(sq[:], yp[:], ysl, ALU.mult)
                        nc.vector.tensor_reduce(
                            out=sqs[:, 0, mh, b, g:g + 1], in_=sq[:],
                            axis=mybir.AxisListType.X, op=ALU.add)

            # =================== BN1 stats allreduce ===================
            for col, buf in ((0, sums), (2, sqs)):
                for mh in range(2):
                    nc.vector.tensor_reduce(
                        out=cc_sb[0][:, col + mh:col + mh + 1],
                        in_=buf[:, 0, mh, :, :], axis=mybir.AxisListType.XY,
                        op=ALU.add)
            nc.sync.dma_start(cc_in[0][:], cc_sb[0][:])
            nc.gpsimd.collective_compute(
                "AllReduce", ALU.add, ins=[cc_in[0][:]], outs=[cc_out[0][:]],
                replica_groups=[list(range(ncores_cc))])
            st0 = const.tile([128, 4], F32, tag="st0")
            nc.sync.dma_start(st0[:], cc_out[0][:])

            def make_scale_bias(st, layer, sv, bv):
                gam = bnp_sb[:, 4 * layer:4 * layer + 2]
                bet = bnp_sb[:, 4 * layer + 2:4 * layer + 4]
                mean = smpool.tile([128, 2], F32, tag="mean")
                nc.vector.tensor_scalar_mul(mean[:], st[:, 0:2], 1.0 / NTOT)
                ex2 = smpool.tile([128, 2], F32, tag="ex2")
                nc.vector.tensor_scalar_mul(ex2[:], st[:, 2:4], 1.0 / NTOT)
                var = smpool.tile([128, 2], F32, tag="var")
                nc.vector.tensor_tensor(var[:], mean[:], mean[:], ALU.mult)
                nc.vector.tensor_tensor(var[:], ex2[:], var[:], ALU.subtract)
                nc.vector.tensor_scalar_add(var[:], var[:], BN_EPS)
                sd = smpool.tile([128, 2], F32, tag="sd")
                nc.scalar.activation(sd[:], var[:], AF.Sqrt)
                istd = smpool.tile([128, 2], F32, tag="istd")
                nc.vector.reciprocal(istd[:], sd[:])
                nc.vector.tensor_tensor(sv[:], gam, istd[:], ALU.mult)
                mb = smpool.tile([128, 2], F32, tag="mb")
                nc.vector.tensor_tensor(mb[:], mean[:], sv[:], ALU.mult)
                nc.vector.tensor_tensor(bv[:], bet, mb[:], ALU.subtract)

            make_scale_bias(st0, 0, svec[0], bvec[0])

            # =================== PHASE B ===================
            for b in range(BPC):
                for g in range(NG):
                    ht = hpool.tile([128, 2, 512], F32R, tag="ht")
                    for kt in range(2):
                        nc.scalar.activation(
                            ht[:, kt, :], y1_sb[b][:, kt, 512 * g:512 * (g + 1)],
                            AF.Relu, bias=bvec[0][:, kt:kt + 1],
                            scale=svec[0][:, kt:kt + 1])
                    for mh in range(2):
                        yp = y_ps.tile([128, 512], F32, tag="yp")
                        for kt in range(2):
                            nc.tensor.matmul(
                                yp[:], w2t_sb[:, kt, mh * 128:(mh + 1) * 128],
                                ht[:, kt, :], start=(kt == 0), stop=(kt == 1))
                        y2s = opool.tile([128, 512], F32, tag="y2s")
                        nc.scalar.activation(y2s[:], yp[:], AF.Copy)
                        nc.vector.tensor_reduce(
                            out=sums[:, 1, mh, b, g:g + 1], in_=y2s[:],
                            axis=mybir.AxisListType.X, op=ALU.add)
                        sq = sqpool.tile([128, 512], F32, tag="sq")
                        nc.vector.tensor_tensor(sq[:], yp[:], y2s[:], ALU.mult)
                        nc.vector.tensor_reduce(
                            out=sqs[:, 1, mh, b, g:g + 1], in_=sq[:],
                            axis=mybir.AxisListType.X, op=ALU.add)
                        nc.sync.dma_start(y2d[b][mh, g], y2s[:])

            # =================== BN2 stats allreduce ===================
            for col, buf in ((0, sums), (2, sqs)):
                for mh in range(2):
                    nc.vector.tensor_reduce(
                        out=cc_sb[1][:, col + mh:col + mh + 1],
                        in_=buf[:, 1, mh, :, :], axis=mybir.AxisListType.XY,
                        op=ALU.add)
            nc.sync.dma_start(cc_in[1][:], cc_sb[1][:])
            nc.gpsimd.collective_compute(
                "AllReduce", ALU.add, ins=[cc_in[1][:]], outs=[cc_out[1][:]],
                replica_groups=[list(range(ncores_cc))])
            st1 = const.tile([128, 4], F32, tag="st1")
            nc.sync.dma_start(st1[:], cc_out[1][:])
            make_scale_bias(st1, 1, svec[1], bvec[1])

            # =================== PHASE C ===================
            for b in range(BPC):
                for g in range(NG):
                    for mh in range(2):
                        yt = opool.tile([128, 512], F32, tag="yt")
                        nc.sync.dma_start(yt[:], y2d[b][mh, g])
                        ot = opool.tile([128, 512], F32, tag="ot")
                        nc.scalar.activation(
                            ot[:], yt[:], AF.Relu, bias=bvec[1][:, mh:mh + 1],
                            scale=svec[1][:, mh:mh + 1])
                        nc.sync.dma_start(
                            out[b][128 * mh:128 * (mh + 1), 512 * g:512 * (g + 1)],
                            ot[:])
    nc.finalize()
    _NC_CACHE["nc"] = nc
    return nc


def _round12(x):
    """Truncate f32 mantissa to 12 bits (safely below f32r's precision)."""
    xi = x.astype(np.float32).view(np.uint32)
    return (xi & np.uint32(0xFFFFE000)).view(np.float32)


def _host_prep(xyz1, xyz2, points1, points2, W1, gamma1, beta1, W2, gamma2, beta2):
    """Builds per-core input maps."""
    xyz1 = np.asarray(xyz1, np.float32)
    xyz2 = np.asarray(xyz2, np.float32)
    points1 = np.asarray(points1, np.float32)
    points2 = np.asarray(points2, np.float32)
    W1 = np.asarray(W1, np.float32)
    W2 = np.asarray(W2, np.float32)
    gamma1 = np.asarray(gamma1, np.float32)
    beta1 = np.asarray(beta1, np.float32)
    gamma2 = np.asarray(gamma2, np.float32)
    beta2 = np.asarray(beta2, np.float32)

    # augmented dist operands, exact split-precision
    h1 = _round12(xyz1)                      # (B,3,N1) high bits
    l1 = (xyz1 - h1).astype(np.float32)
    h2 = _round12(xyz2)
    l2 = (xyz2 - h2).astype(np.float32)
    n1sq = (xyz1.astype(np.float64) ** 2).sum(1).astype(np.float32)   # (B,N1)
    n1h = _round12(n1sq)
    n1l = (n1sq - n1h).astype(np.float32)
    n2sq = (xyz2.astype(np.float64) ** 2).sum(1).astype(np.float32)
    n2h = _round12(n2sq)
    n2l = (n2sq - n2h).astype(np.float32)

    x1aug = np.zeros((B, KAUG, N1), np.float32)
    x1aug[:, 0:3] = h1
    x1aug[:, 3:6] = h1
    x1aug[:, 6:9] = l1
    x1aug[:, 9] = n1h
    x1aug[:, 10] = n1l
    x1aug[:, 11] = 1.0
    x1aug[:, 12] = 1.0
    x1aug[:, 13:16] = l1

    x2aug = np.zeros((B, KAUG, N2), np.float32)
    x2aug[:, 0:3] = 2.0 * h2
    x2aug[:, 3:6] = 2.0 * l2
    x2aug[:, 6:9] = 2.0 * h2
    x2aug[:, 9] = -1.0
    x2aug[:, 10] = -1.0
    x2aug[:, 11] = -n2h
    x2aug[:, 12] = -n2l
    x2aug[:, 13:16] = 2.0 * l2

    p2t = np.ascontiguousarray(points2.transpose(0, 2, 1))     # (B, N2, C2)

    w1t = np.ascontiguousarray(
        W1.T.reshape(3, 128, CH).transpose(1, 0, 2))           # (128,3,CH)
    w2t = np.ascontiguousarray(
        W2.T.reshape(2, 128, COUT).transpose(1, 0, 2))         # (128,2,COUT)

    bnp = np.zeros((128, 8), np.float32)
    bnp[:, 0] = gamma1[0:128]
    bnp[:, 1] = gamma1[128:256]
    bnp[:, 2] = beta1[0:128]
    bnp[:, 3] = beta1[128:256]
    bnp[:, 4] = gamma2[0:128]
    bnp[:, 5] = gamma2[128:256]
    bnp[:, 6] = beta2[0:128]
    bnp[:, 7] = beta2[128:256]

    ident = np.eye(128, dtype=np.float32)

    in_maps = []
    for c in range(NCORES):
        bs = slice(c * BPC, (c + 1) * BPC)
        in_maps.append({
            "x1aug": np.ascontiguousarray(x1aug[bs]),
            "x2aug": np.ascontiguousarray(x2aug[bs]),
            "p1": np.ascontiguousarray(points1[bs]),
            "p2t": np.ascontiguousarray(p2t[bs]),
            "w1t": w1t, "w2t": w2t, "identw": ident, "bnp": bnp,
        })
    return in_maps


def kernel(**inputs):
    nc = build()
    in_maps = _host_prep(**inputs)
    res = run_bass_kernel_spmd(nc, in_maps, list(range(NCORES)))
    out = np.empty((B, COUT, N1), np.float32)
    for c in range(NCORES):
        out[c * BPC:(c + 1) * BPC] = res.results[c]["out"]
    return out


if __name__ == "__main__":
    rng = np.random.default_rng(0)
    ins = {
        "xyz1": rng.standard_normal((B, 3, N1), np.float32),
        "xyz2": rng.standard_normal((B, 3, N2), np.float32),
        "points1": rng.standard_normal((B, C1, N1), np.float32),
        "points2": rng.standard_normal((B, C2, N2), np.float32),
        "W1": rng.standard_normal((CH, CIN), np.float32) / np.sqrt(CIN),
        "gamma1": np.ones(CH, np.float32),
        "beta1": np.zeros(CH, np.float32),
        "W2": rng.standard_normal((COUT, CH), np.float32) / np.sqrt(CH),
        "gamma2": np.ones(COUT, np.float32),
        "beta2": np.zeros(COUT, np.float32),
    }
    o = kernel(**ins)
    print("out", o.shape, o.dtype, float(np.abs(o).mean()))
